# revision 1
# baseline (speedup 1.0000x reference)
"""Trainium2 Bass kernel for ragged-sequence gather:

    out[pid] = verified_id[num_draft_tokens * pid + accept_lens[pid] - 1]

with BS = 2_097_152 groups, num_draft_tokens = 16, verified_id fp32 of
shape [BS*16], accept_lens int64 of shape [BS] with values in [1, 16].

Strategy (8 NeuronCores, batch-sharded):
  - Core c owns groups [c*BS/8, (c+1)*BS/8): a contiguous 16 MiB slice of
    verified_id, a 1 MiB (int32) slice of accept_lens, and writes a 1 MiB
    output slice.  Fully local, no collectives.
  - On-chip, verified data is streamed as [128, F] tiles (each group of 16
    lies contiguously inside a partition row).  A custom DVE op (SEL16)
    computes  prod[p,g,k] = V[p,g,k] * (k+1 == lens[p,g])  in a single 1x
    pass using the DVE's Idx/PageIdx hardware counters (page size 16), with
    lens broadcast via a stride-0 access pattern - no iota constant, no
    mask materialization.  A segmented tensor_reduce(add) then collapses
    each group of 16 (exact: 15 zeros + the selected value), and the result
    is DMA'd out.
"""

import sys

import numpy as np

if "/opt/trn_rl_repo" not in sys.path:
    sys.path.insert(0, "/opt/trn_rl_repo")

P = 128
ND = 16
BS = 2_097_152
N_CORES = 8
G_CORE = BS // N_CORES              # groups per core = 262144
FD_CORE = G_CORE * ND // P          # fp32 elems per partition = 32768
G_P = G_CORE // P                   # groups per partition = 2048

_SEL16_NAME = "ANT_SELECT16_V1"
_sel16_op = None


def _get_sel16():
    """Build + register the custom DVE op at runtime (appended to OPS).

    body: out[k] = select(Idx + (1 - 16*page) == Src1, Src0, 0)
    With in0 = V as [P, S, 16] and in1 = lens (f32) broadcast [P, S, 16],
    Idx is the global element counter and PageIdx(One, s0=-16) holds
    1 - 16*s within page s, so Idx + pg = (k_within_page + 1) in [1, 16].
    """
    global _sel16_op
    if _sel16_op is not None:
        return _sel16_op
    from concourse import dve_ops as dvo
    from concourse.dve_spec import (
        Spec, Src0, Src1, C0, Zero, One, eq, select, PageIdx, Idx, lower,
    )
    from concourse.dve_uop import DveOpSpec

    pg = PageIdx(One, C0)            # 1 + s*c0, call with s0 = -16.0
    body = select(eq(Idx + pg, Src1), Src0, Zero)

    def _ref(in0, in1, c0, c1, c2):
        a = np.asarray(in0, np.float32)
        l = np.asarray(in1, np.float32)
        p = a.shape[0]
        a3 = a.reshape(p, -1, ND)
        l3 = np.broadcast_to(l.reshape(p, -1, ND) if l.size == a.size
                             else l.reshape(p, -1, 1), a3.shape)
        s = a3.shape[1]
        gidx = np.arange(s * ND, dtype=np.float32).reshape(1, s, ND)
        pgv = 1.0 + np.arange(s, dtype=np.float32).reshape(1, s, 1) * float(c0)
        mask = (gidx + pgv) == l3
        return np.where(mask, a3, np.float32(0.0)).reshape(a.shape)

    spec = Spec(body=body, reference=_ref)
    shas = {}
    for ver in ("v3", "v4"):
        try:
            uops = lower(spec, ver=ver)
            shas[ver] = DveOpSpec(
                name=_SEL16_NAME, opcode=1, uops=uops, rd1_en=True
            ).sha(ver)
        except Exception:
            pass

    op = dvo.DveOp(_SEL16_NAME, spec, subdim=True, uops_sha=shas)
    if _SEL16_NAME not in dvo._SUB_OPCODE_FOR_NAME:
        dvo.OPS.append(op)
        row = dvo._CUSTOM_DVE_ROW_BASE + len(dvo.OPS) - 1
        assert row < 0x20
        dvo._SUB_OPCODE_FOR_NAME[_SEL16_NAME] = row
        dvo.CUSTOM_DVE_SPECS[_SEL16_NAME] = spec
    _sel16_op = op
    return op


def build_bass(fd_p=FD_CORE, nt=0, ramp=2, tail_ramp=0, gp=0, lens_cast=0, sched=0, vb=4, ldma=1):
    """Build the per-core Bass program.

    fd_p: total fp32 elements per partition (divisible by nt*16)
    nt:   number of full-size tiles the bulk is split into
    ramp: split the first tile into `ramp` sub-tiles for a faster pipeline
          warm-up (0/1 = disabled)
    tail_ramp: split the last tile into `tail_ramp` sub-tiles so the final
          output DMA shrinks (0/1 = disabled)
    gp:   number of full-size tiles whose select is computed on GPSIMD via
          the stock mask pipeline (ACT expansion + eq + mult) instead of the
          DVE custom op, to offload the DVE bottleneck
    """
    import concourse.bacc as bacc
    import concourse.mybir as mybir
    import ml_dtypes
    from concourse.tile import TileContext

    f32 = mybir.dt.float32
    i32 = mybir.dt.int32
    bf16 = mybir.dt.bfloat16

    fdt = fd_p // nt if nt else fd_p
    assert fdt % ND == 0 and fdt * (nt or 1) == fd_p
    g_p = fd_p // ND

    # tile schedule: (elem offset, elems) per partition
    if nt == 0:
        # mixed schedule: small tiles to ramp the pipeline, 4096-elem tiles
        # for the overhead-amortized steady phase, 2048 tail
        if sched == 2:
            sizes = [1024] * 2 + [2048] * 2 + [4096] * 6 + [1024] + [512] * 2
        elif sched:
            sizes = [1024] * 2 + [2048] * 3 + [4096] * 5 + [2048] * 2
        else:
            sizes = [1024] * 2 + [2048] * 2 + [4096] * 6 + [2048]
        assert sum(sizes) == fd_p
        tiles, off0 = [], 0
        for s in sizes:
            tiles.append((off0, s))
            off0 += s
    else:
        tiles = [(t * fdt, fdt) for t in range(nt)]
        if ramp and ramp > 1 and fdt % (ramp * ND) == 0:
            sub = fdt // ramp
            tiles[0:1] = [(i * sub, sub) for i in range(ramp)]
        if tail_ramp and tail_ramp > 1 and fdt % (tail_ramp * ND) == 0:
            off0 = tiles[-1][0]
            sub = fdt // tail_ramp
            tiles[-1:] = [(off0 + i * sub, sub) for i in range(tail_ramp)]

    sel16 = _get_sel16()

    nc = bacc.Bacc("TRN2", target_bir_lowering=False)

    v_d = nc.dram_tensor("v", [P, fd_p], f32, kind="ExternalInput")
    l_d = nc.dram_tensor("lens", [P, g_p], i32, kind="ExternalInput")
    o_d = nc.dram_tensor("o", [P, g_p], f32, kind="ExternalOutput")

    # which tile indices run on GPSIMD: spread through the middle fulls
    full_idx = [i for i, (_, n) in enumerate(tiles) if n == fdt]
    gp_set = set(full_idx[1:1 + gp]) if gp else set()

    iota_d = None
    if gp_set:
        iota_np = np.tile(np.arange(1, ND + 1, dtype=np.float32), fdt // ND)
        iota_np = np.ascontiguousarray(
            iota_np.astype(ml_dtypes.bfloat16).reshape(1, fdt)
        )
        iota_d = nc.inline_tensor(iota_np, name="iota1_const")

    with TileContext(nc) as tc:
        with tc.tile_pool(name="work", bufs=3) as pool:
            if gp_set:
                iota_t = pool.tile([P, fdt], bf16, tag="iota", bufs=1)
                nc.gpsimd.dma_start(
                    out=iota_t[:], in_=iota_d[0:1, :].partition_broadcast(P)
                )
            for i, (off, n) in enumerate(tiles):
                goff, gn = off // ND, n // ND
                vt = pool.tile([P, n], f32, tag=f"v{n}", bufs=(vb if n == 4096 else 2 if n <= 1024 else 3))
                nc.sync.dma_start(out=vt[:], in_=v_d[:, off:off + n])
                lt = pool.tile([P, gn], i32, tag=f"l{n}")
                # ldma=1: lens via SWDGE so it never queues behind V tiles
                # on the HWDGE rings during the ramp
                (nc.gpsimd if ldma else nc.sync).dma_start(
                    out=lt[:], in_=l_d[:, goff:goff + gn])

                if lens_cast:
                    # int32 -> fp32 cast on the (otherwise idle) ACT engine
                    lf = pool.tile([P, gn], f32, tag=f"lf{n}")
                    nc.scalar.copy(out=lf[:], in_=lt[:])
                else:
                    # DVE read port converts int32 -> fp32 internally
                    lf = lt

                prod = pool.tile([P, n], f32, tag=f"prod{n}", bufs=2)
                if i in gp_set:
                    # GPSIMD pipeline: ACT expands lens to bf16, GPSIMD does
                    # eq + mult (frees the DVE for other tiles)
                    lexp = pool.tile([P, n], bf16, tag="lexp", bufs=2)
                    nc.scalar.copy(
                        out=lexp[:].rearrange("p (g k) -> p g k", k=ND),
                        in_=lf[:, :, None].to_broadcast([P, gn, ND]),
                    )
                    # eq on DVE (2x bf16), mult on GPSIMD (Pool rejects the
                    # BITVEC is_equal opcode but supports ARITH mult)
                    mask = pool.tile([P, n], bf16, tag="mask", bufs=2)
                    nc.vector.tensor_tensor(
                        out=mask[:], in0=lexp[:], in1=iota_t[:],
                        op=mybir.AluOpType.is_equal,
                    )
                    nc.gpsimd.tensor_tensor(
                        out=prod[:], in0=mask[:], in1=vt[:],
                        op=mybir.AluOpType.mult,
                    )
                else:
                    nc.vector._custom_dve(
                        sel16,
                        out=prod[:].rearrange("p (g k) -> p g k", k=ND),
                        in0=vt[:].rearrange("p (g k) -> p g k", k=ND),
                        in1=lf[:, :, None].to_broadcast([P, gn, ND]),
                        s0=-float(ND),
                    )

                ot = pool.tile([P, gn], f32, tag=f"o{n}")
                nc.vector.tensor_reduce(
                    out=ot[:],
                    in_=prod[:].rearrange("p (g k) -> p g k", k=ND),
                    axis=mybir.AxisListType.X,
                    op=mybir.AluOpType.add,
                )
                nc.sync.dma_start(out=o_d[:, goff:goff + gn], in_=ot[:])
    if not nc.is_finalized():
        nc.finalize()
    return nc


_CACHE = {}


def _get_nc(**kw):
    key = tuple(sorted(kw.items()))
    if key not in _CACHE:
        _CACHE[key] = build_bass(**kw)
    return _CACHE[key]


def kernel(verified_id, accept_lens, num_draft_tokens, **run_kw):
    from concourse.bass_utils import run_bass_kernel_spmd

    assert int(num_draft_tokens) == ND
    v = np.ascontiguousarray(np.asarray(verified_id, dtype=np.float32))
    lens = np.asarray(accept_lens)
    assert v.shape == (BS * ND,) and lens.shape == (BS,)
    l32 = np.ascontiguousarray(lens.astype(np.int32))

    v3 = v.reshape(N_CORES, P, FD_CORE)
    l3 = l32.reshape(N_CORES, P, G_P)

    nc = _get_nc()
    in_maps = [{"v": v3[c], "lens": l3[c]} for c in range(N_CORES)]
    res = run_bass_kernel_spmd(nc, in_maps, core_ids=list(range(N_CORES)), **run_kw)
    out = np.stack([res.results[c]["o"] for c in range(N_CORES)])
    ret = out.reshape(-1)
    if run_kw:
        return ret, res
    return ret



# revision 2
# speedup vs baseline: 1.2230x; 1.2230x over previous
"""Trainium2 Bass kernel for ragged-sequence gather:

    out[pid] = verified_id[num_draft_tokens * pid + accept_lens[pid] - 1]

with BS = 2_097_152 groups, num_draft_tokens = 16, verified_id fp32 of
shape [BS*16], accept_lens int64 of shape [BS] with values in [1, 16].

Strategy (8 NeuronCores, batch-sharded):
  - Core c owns groups [c*BS/8, (c+1)*BS/8): a contiguous 16 MiB slice of
    verified_id, a slice of accept_lens, and writes a 1 MiB output slice.
    Fully local, no collectives.
  - On-chip, verified data streams as [128, F] fp32 tiles (each group of 16
    lies contiguously inside a partition row).  A hand-built custom DVE op
    (SELHOLD16) fuses select + per-group emission into a single 1x pass:
    a 4-uOp FSM cycles FIRST -> STEADY(x14) -> EMIT per 16-element group.
    Stage 0 counts k (reset to 1 in FIRST), stage 1 compares k == len,
    stage 2 SELECTs between the incoming value (on match) and its own
    previous output (temporal CURR_ALU_OUT feedback = hold last selected).
    Only the EMIT uOp enables the write port, so each group of 16 inputs
    emits exactly one fp32: out directly has the gathered shape [P, G].
    One DVE instruction per tile, ~1 elem/cycle; DVE time ~34 us/core,
    under the ~53 us/core HBM DMA floor, so the kernel is DMA-bound.
"""

import sys

import numpy as np

if "/opt/trn_rl_repo" not in sys.path:
    sys.path.insert(0, "/opt/trn_rl_repo")

P = 128
ND = 16
BS = 2_097_152
N_CORES = 8
G_CORE = BS // N_CORES              # groups per core = 262144
FD_CORE = G_CORE * ND // P          # fp32 elems per partition = 32768
G_P = G_CORE // P                   # groups per partition = 2048

_SELHOLD_NAME = "ANT_SELHOLD16_V1"
_selhold_op = None


def _build_selhold_uops():
    """The 4-uOp FSM implementing the fused gather.

    inputs: lane0 <- ONE_F32 (feeds stage0 as PREV_ALU_OUT)
            lane1 (delay 0) <- SRC_0 = v values
            lane2 (delay 1) <- SRC_1 = accept len (read-port converts to f32)
    stage0: k counter.  FIRST: k := 1 (BYPASS 1.0); else k := CURR + 1.
    stage1: IS_EQ(k, len)
    stage2: SELECT(cond=prev, true -> v, false -> CURR_ALU_OUT) = hold
    stage3-7: BYPASS carries the running value to the output mux.
    Only EMIT (every 16th element) has the write port enabled.
    """
    from concourse.dve_uop import (
        ENABLE,
        AluInp,
        AluOp,
        InpSel,
        OutPath,
        OutSel,
        Trigger,
        UopConfig,
    )

    def mk(first: bool, emit: bool) -> UopConfig:
        u = UopConfig()
        u.enable_input(InpSel.ONE_F32, 0)
        u.enable_input(InpSel.SRC_0, 1)
        u.enable_input(InpSel.SRC_1, 2)
        dp = u.datapath_config
        if first:
            dp[0].enable_alu(AluOp.BYPASS, AluInp.PREV_ALU_OUT)
        else:
            dp[0].enable_alu(AluOp.ADD, AluInp.CURR_ALU_OUT, AluInp.PREV_ALU_OUT)
        dp[0].pass_through_delay(0, 1)
        dp[1].enable_alu(
            AluOp.IS_EQ, AluInp.PREV_ALU_OUT, AluInp.PREV_DELAY_1
        ).pass_through_delay(0)
        # SELECT: cond arrives implicitly via PREV_ALU_OUT (the IS_EQ result);
        # alu_src0 = false value (hold own previous output), alu_src1 = true
        # value (the selected v).
        dp[2].enable_alu(AluOp.SELECT, AluInp.CURR_ALU_OUT, AluInp.PREV_DELAY_0)
        for k in range(3, 8):
            dp[k].pass_through_alu()
        u.require_inp0 = ENABLE
        u.require_inp1 = ENABLE
        if emit:
            u.enable_output(OutSel.ALU_OUT, OutPath.WR0_LO)
        return u

    CN = (Trigger.COUNT, Trigger.NONE, Trigger.NONE)
    u0 = UopConfig()                      # entry dead cycle (uop 0 = IDLE target)
    u0.repeat_count, u0.trigger, u0.next_uop = 1, CN, (1, 0, 0)
    u1 = mk(True, False)                  # FIRST: k := 1
    u1.repeat_count, u1.trigger, u1.next_uop = 1, CN, (2, 0, 0)
    u2 = mk(False, False)                 # STEADY x14
    u2.repeat_count, u2.trigger, u2.next_uop = 14, CN, (3, 0, 0)
    u3 = mk(False, True)                  # EMIT (writes), loop or finish
    u3.repeat_count = 1
    u3.trigger = (Trigger.SRC_TENSOR_DONE, Trigger.COUNT, Trigger.NONE)
    u3.next_uop = (0, 1, 0)
    return [u0, u1, u2, u3]


def _ref_selhold(in0, in1, c0, c1, c2):
    a = np.asarray(in0, np.float32)
    p = a.shape[0]
    a3 = a.reshape(p, -1, ND)
    l = np.asarray(in1)
    l3 = l.reshape(p, -1, ND)[:, :, 0] if l.size == a.size else l.reshape(p, -1)
    idx = np.clip(l3.astype(np.int64) - 1, 0, ND - 1)
    return np.take_along_axis(a3, idx[:, :, None], axis=2)[:, :, 0]


def _get_selhold():
    """Build + register the custom DVE op (appended to dve_ops.OPS)."""
    global _selhold_op
    if _selhold_op is not None:
        return _selhold_op
    from concourse import dve_ops as dvo
    from concourse.dve_spec import Spec, Src0, Src1, Zero, eq, select, Idx
    from concourse.dve_uop import DveOpSpec

    # Representative Spec: reads Src0+Src1, no C2 (so the STT-shape check in
    # _custom_dve passes); `reference` carries the true semantics for interp.
    spec = Spec(body=select(eq(Idx, Src1), Src0, Zero), reference=_ref_selhold)
    uops = _build_selhold_uops()

    class RawDveOp(dvo.DveOp):
        """DveOp whose table program is hand-built, not lower()ed from spec."""

        def __init__(self, name, spec, subdim, raw_uops):
            object.__setattr__(self, "name", name)
            object.__setattr__(self, "spec", spec)
            object.__setattr__(self, "subdim", subdim)
            object.__setattr__(self, "uops_sha", {})
            object.__setattr__(self, "perf_en", {})
            object.__setattr__(self, "_raw_uops", raw_uops)

        def compile(self, ver):
            key = (self.name, ver)
            if key not in dvo._COMPILE_CACHE:
                r = DveOpSpec(
                    name=self.name,
                    opcode=dvo.get_dve_sub_opcode(self.name),
                    uops=self._raw_uops,
                    rd1_en=True,
                )
                r.validate(ver)
                dvo._COMPILE_CACHE[key] = r
            return dvo._COMPILE_CACHE[key]

    op = RawDveOp(_SELHOLD_NAME, spec, True, uops)
    if _SELHOLD_NAME not in dvo._SUB_OPCODE_FOR_NAME:
        dvo.OPS.append(op)
        row = dvo._CUSTOM_DVE_ROW_BASE + len(dvo.OPS) - 1
        assert row < 0x20
        dvo._SUB_OPCODE_FOR_NAME[_SELHOLD_NAME] = row
        dvo.CUSTOM_DVE_SPECS[_SELHOLD_NAME] = spec
    _selhold_op = op
    return op


def build_bass(fd_p=FD_CORE, sched=0, vb=4, lens_i8=0, odma=1, ldma=1):
    """Build the per-core Bass program.

    fd_p:    total fp32 elements per partition
    sched:   tile-size schedule selector
    vb:      bufs for the steady-state (4096-elem) v tiles
    lens_i8: read accept_lens as int8 (1 B/group) instead of int32
    odma:    0 = output DMA on sync ring, 1 = on scalar (ACT) HWDGE ring
    ldma:    1 = lens DMA via gpsimd SWDGE (doesn't queue behind v tiles)
    """
    import concourse.bacc as bacc
    import concourse.mybir as mybir
    from concourse.tile import TileContext

    f32 = mybir.dt.float32
    ldt = mybir.dt.int8 if lens_i8 else mybir.dt.int32
    g_p = fd_p // ND

    if sched == 0:
        sizes = [1024] * 2 + [2048] * 2 + [4096] * 6 + [2048]
    elif sched == 1:
        sizes = [512, 512, 1024, 2048] + [4096] * 6 + [2048, 2048]
    else:
        sizes = [2048] * 2 + [4096] * 7 + [512]
    assert sum(sizes) == fd_p
    tiles, off0 = [], 0
    for s in sizes:
        tiles.append((off0, s))
        off0 += s

    selhold = _get_selhold()

    nc = bacc.Bacc("TRN2", target_bir_lowering=False)

    v_d = nc.dram_tensor("v", [P, fd_p], f32, kind="ExternalInput")
    l_d = nc.dram_tensor("lens", [P, g_p], ldt, kind="ExternalInput")
    o_d = nc.dram_tensor("o", [P, g_p], f32, kind="ExternalOutput")

    with TileContext(nc) as tc:
        with tc.tile_pool(name="work", bufs=3) as pool:
            for i, (off, n) in enumerate(tiles):
                goff, gn = off // ND, n // ND
                vt = pool.tile(
                    [P, n], f32, tag=f"v{n}",
                    bufs=(vb if n == 4096 else 2 if n <= 1024 else 3),
                )
                nc.sync.dma_start(out=vt[:], in_=v_d[:, off:off + n])
                lt = pool.tile([P, gn], ldt, tag=f"l{n}")
                (nc.gpsimd if ldma else nc.sync).dma_start(
                    out=lt[:], in_=l_d[:, goff:goff + gn])

                ot = pool.tile([P, gn], f32, tag=f"o{n}")
                nc.vector._custom_dve(
                    selhold,
                    out=ot[:],
                    in0=vt[:].rearrange("p (g k) -> p g k", k=ND),
                    in1=lt[:, :, None].to_broadcast([P, gn, ND]),
                )
                (nc.scalar if odma else nc.sync).dma_start(
                    out=o_d[:, goff:goff + gn], in_=ot[:])
    if not nc.is_finalized():
        nc.finalize()
    return nc


_CACHE = {}


def _get_nc(**kw):
    key = tuple(sorted(kw.items()))
    if key not in _CACHE:
        _CACHE[key] = build_bass(**kw)
    return _CACHE[key]


def kernel(verified_id, accept_lens, num_draft_tokens, **run_kw):
    from concourse.bass_utils import run_bass_kernel_spmd

    assert int(num_draft_tokens) == ND
    v = np.ascontiguousarray(np.asarray(verified_id, dtype=np.float32))
    lens = np.asarray(accept_lens)
    assert v.shape == (BS * ND,) and lens.shape == (BS,)

    build_kw = dict(BUILD_KW)
    if build_kw.get("lens_i8"):
        l_np = np.ascontiguousarray(lens.astype(np.int8))
    else:
        l_np = np.ascontiguousarray(lens.astype(np.int32))

    v3 = v.reshape(N_CORES, P, FD_CORE)
    l3 = l_np.reshape(N_CORES, P, G_P)

    nc = _get_nc(**build_kw)
    in_maps = [{"v": v3[c], "lens": l3[c]} for c in range(N_CORES)]
    res = run_bass_kernel_spmd(nc, in_maps, core_ids=list(range(N_CORES)), **run_kw)
    out = np.stack([res.results[c]["o"] for c in range(N_CORES)])
    ret = out.reshape(-1)
    if run_kw:
        return ret, res
    return ret


BUILD_KW = {}


# revision 7
# speedup vs baseline: 1.3332x; 1.0901x over previous
"""Trainium2 Bass kernel for ragged-sequence gather:

    out[pid] = verified_id[num_draft_tokens * pid + accept_lens[pid] - 1]

with BS = 2_097_152 groups, num_draft_tokens = 16, verified_id fp32 of
shape [BS*16], accept_lens int64 of shape [BS] with values in [1, 16].

Strategy (8 NeuronCores, batch-sharded):
  - Core c owns groups [c*BS/8, (c+1)*BS/8): a contiguous 16 MiB slice of
    verified_id, a slice of accept_lens, and writes a 1 MiB output slice.
    Fully local, no collectives.
  - On-chip, verified data streams as [128, F] fp32 tiles (each group of 16
    lies contiguously inside a partition row).  A hand-built custom DVE op
    (SELHOLD16) fuses select + per-group emission into a single 1x pass:
    a 4-uOp FSM cycles FIRST -> STEADY(x14) -> EMIT per 16-element group.
    Stage 0 counts k (reset to 1 in FIRST), stage 1 compares k == len,
    stage 2 SELECTs between the incoming value (on match) and its own
    previous output (temporal CURR_ALU_OUT feedback = hold last selected).
    Only the EMIT uOp enables the write port, so each group of 16 inputs
    emits exactly one fp32: out directly has the gathered shape [P, G].
    One DVE instruction per tile, ~1 elem/cycle; DVE time ~34 us/core,
    under the ~53 us/core HBM DMA floor, so the kernel is DMA-bound.
"""

import sys

import numpy as np

if "/opt/trn_rl_repo" not in sys.path:
    sys.path.insert(0, "/opt/trn_rl_repo")

P = 128
ND = 16
BS = 2_097_152
N_CORES = 8
G_CORE = BS // N_CORES              # groups per core = 262144
FD_CORE = G_CORE * ND // P          # fp32 elems per partition = 32768
G_P = G_CORE // P                   # groups per partition = 2048

_SELHOLD_NAME = "ANT_SELHOLD16_V1"
_selhold_op = None


def _build_selhold_uops():
    """The 4-uOp FSM implementing the fused gather.

    inputs: lane0 <- ONE_F32 (feeds stage0 as PREV_ALU_OUT)
            lane1 (delay 0) <- SRC_0 = v values
            lane2 (delay 1) <- SRC_1 = accept len (read-port converts to f32)
    stage0: k counter.  FIRST: k := 1 (BYPASS 1.0); else k := CURR + 1.
    stage1: IS_EQ(k, len)
    stage2: SELECT(cond=prev, true -> v, false -> CURR_ALU_OUT) = hold
    stage3-7: BYPASS carries the running value to the output mux.
    Only EMIT (every 16th element) has the write port enabled.
    """
    from concourse.dve_uop import (
        ENABLE,
        AluInp,
        AluOp,
        InpSel,
        OutPath,
        OutSel,
        Trigger,
        UopConfig,
    )

    def mk(first: bool, emit: bool) -> UopConfig:
        u = UopConfig()
        u.enable_input(InpSel.ONE_F32, 0)
        u.enable_input(InpSel.SRC_0, 1)
        u.enable_input(InpSel.SRC_1, 2)
        dp = u.datapath_config
        if first:
            dp[0].enable_alu(AluOp.BYPASS, AluInp.PREV_ALU_OUT)
        else:
            dp[0].enable_alu(AluOp.ADD, AluInp.CURR_ALU_OUT, AluInp.PREV_ALU_OUT)
        dp[0].pass_through_delay(0, 1)
        dp[1].enable_alu(
            AluOp.IS_EQ, AluInp.PREV_ALU_OUT, AluInp.PREV_DELAY_1
        ).pass_through_delay(0)
        # SELECT: cond arrives implicitly via PREV_ALU_OUT (the IS_EQ result);
        # alu_src0 = false value (hold own previous output), alu_src1 = true
        # value (the selected v).
        dp[2].enable_alu(AluOp.SELECT, AluInp.CURR_ALU_OUT, AluInp.PREV_DELAY_0)
        for k in range(3, 8):
            dp[k].pass_through_alu()
        u.require_inp0 = ENABLE
        u.require_inp1 = ENABLE
        if emit:
            u.enable_output(OutSel.ALU_OUT, OutPath.WR0_LO)
        return u

    CN = (Trigger.COUNT, Trigger.NONE, Trigger.NONE)
    u0 = UopConfig()                      # entry dead cycle (uop 0 = IDLE target)
    u0.repeat_count, u0.trigger, u0.next_uop = 1, CN, (1, 0, 0)
    u1 = mk(True, False)                  # FIRST: k := 1
    u1.repeat_count, u1.trigger, u1.next_uop = 1, CN, (2, 0, 0)
    u2 = mk(False, False)                 # STEADY x14
    u2.repeat_count, u2.trigger, u2.next_uop = 14, CN, (3, 0, 0)
    u3 = mk(False, True)                  # EMIT (writes), loop or finish
    u3.repeat_count = 1
    u3.trigger = (Trigger.SRC_TENSOR_DONE, Trigger.COUNT, Trigger.NONE)
    u3.next_uop = (0, 1, 0)
    return [u0, u1, u2, u3]


def _ref_selhold(in0, in1, c0, c1, c2):
    a = np.asarray(in0, np.float32)
    p = a.shape[0]
    a3 = a.reshape(p, -1, ND)
    l = np.asarray(in1)
    l3 = l.reshape(p, -1, ND)[:, :, 0] if l.size == a.size else l.reshape(p, -1)
    idx = np.clip(l3.astype(np.int64) - 1, 0, ND - 1)
    return np.take_along_axis(a3, idx[:, :, None], axis=2)[:, :, 0]


def _get_selhold():
    """Build + register the custom DVE op (appended to dve_ops.OPS)."""
    global _selhold_op
    if _selhold_op is not None:
        return _selhold_op
    from concourse import dve_ops as dvo
    from concourse.dve_spec import Spec, Src0, Src1, Zero, eq, select, Idx
    from concourse.dve_uop import DveOpSpec

    # Representative Spec: reads Src0+Src1, no C2 (so the STT-shape check in
    # _custom_dve passes); `reference` carries the true semantics for interp.
    spec = Spec(body=select(eq(Idx, Src1), Src0, Zero), reference=_ref_selhold)
    uops = _build_selhold_uops()

    class RawDveOp(dvo.DveOp):
        """DveOp whose table program is hand-built, not lower()ed from spec."""

        def __init__(self, name, spec, subdim, raw_uops):
            object.__setattr__(self, "name", name)
            object.__setattr__(self, "spec", spec)
            object.__setattr__(self, "subdim", subdim)
            object.__setattr__(self, "uops_sha", {})
            object.__setattr__(self, "perf_en", {})
            object.__setattr__(self, "_raw_uops", raw_uops)

        def compile(self, ver):
            key = (self.name, ver)
            if key not in dvo._COMPILE_CACHE:
                r = DveOpSpec(
                    name=self.name,
                    opcode=dvo.get_dve_sub_opcode(self.name),
                    uops=self._raw_uops,
                    rd1_en=True,
                )
                r.validate(ver)
                dvo._COMPILE_CACHE[key] = r
            return dvo._COMPILE_CACHE[key]

    op = RawDveOp(_SELHOLD_NAME, spec, True, uops)
    if _SELHOLD_NAME not in dvo._SUB_OPCODE_FOR_NAME:
        dvo.OPS.append(op)
        row = dvo._CUSTOM_DVE_ROW_BASE + len(dvo.OPS) - 1
        assert row < 0x20
        dvo._SUB_OPCODE_FOR_NAME[_SELHOLD_NAME] = row
        dvo.CUSTOM_DVE_SPECS[_SELHOLD_NAME] = spec
    _selhold_op = op
    return op


def build_bass(fd_p=FD_CORE, sched=0, vb=6, lens_i8=1, odma=1, ob=4, lchunks=2):
    """Build the per-core Bass program.

    fd_p:    total fp32 elements per partition
    sched:   tile-size schedule selector
    vb:      bufs for the steady-state (4096-elem) v tiles
    lens_i8: read accept_lens as int8 (1 B/group) instead of int32
    odma:    0 = output DMA on sync ring, 1 = on scalar (ACT) HWDGE ring
    ob:      bufs for output tiles
    lchunks: lens is DMA'd upfront in this many chunks (HWDGE scalar ring;
             no SWDGE anywhere - avoids the SDMA-15 descriptor-ring slowdown)
    """
    import concourse.bacc as bacc
    import concourse.mybir as mybir
    from concourse.tile import TileContext

    f32 = mybir.dt.float32
    ldt = {0: mybir.dt.int32, 1: mybir.dt.int8, 2: mybir.dt.int16,
           3: mybir.dt.uint8, 4: mybir.dt.bfloat16, 5: mybir.dt.float16,
           6: mybir.dt.float8e4,
           }[lens_i8]
    g_p = fd_p // ND

    if sched == 0:
        sizes = [1024] * 2 + [2048] * 2 + [4096] * 6 + [1536, 512]
    elif sched == 1:
        sizes = [512, 512, 1024, 2048] + [4096] * 6 + [2048, 1536, 512]
    else:
        sizes = [1024] * 2 + [2048] * 2 + [4096] * 6 + [2048]
    assert sum(sizes) == fd_p
    tiles, off0 = [], 0
    for s in sizes:
        tiles.append((off0, s))
        off0 += s

    selhold = _get_selhold()

    nc = bacc.Bacc("TRN2", target_bir_lowering=False)

    v_d = nc.dram_tensor("v", [P, fd_p], f32, kind="ExternalInput")
    l_d = nc.dram_tensor("lens", [P, g_p], ldt, kind="ExternalInput")
    o_d = nc.dram_tensor("o", [P, g_p], f32, kind="ExternalOutput")

    # lens chunk boundaries (in groups), aligned to tile group offsets
    gbounds = []
    acc = 0
    per = g_p // lchunks
    for c in range(lchunks):
        acc += per
        gbounds.append(g_p if c == lchunks - 1 else acc)

    with TileContext(nc) as tc:
        with tc.tile_pool(name="work", bufs=3) as pool:
            # whole lens staged upfront in `lchunks` HWDGE transfers
            lt = pool.tile([P, g_p], ldt, tag="lens", bufs=1)
            g0 = 0
            for g1 in gbounds:
                nc.scalar.dma_start(out=lt[:, g0:g1], in_=l_d[:, g0:g1])
                g0 = g1
            for i, (off, n) in enumerate(tiles):
                goff, gn = off // ND, n // ND
                vt = pool.tile(
                    [P, n], f32, tag=f"v{n}",
                    bufs=(vb if n == 4096 else 2 if n <= 1024 else 3),
                )
                nc.sync.dma_start(out=vt[:], in_=v_d[:, off:off + n])

                ot = pool.tile([P, gn], f32, tag=f"o{n}", bufs=ob)
                nc.vector._custom_dve(
                    selhold,
                    out=ot[:],
                    in0=vt[:].rearrange("p (g k) -> p g k", k=ND),
                    in1=lt[:, goff:goff + gn, None].to_broadcast([P, gn, ND]),
                )
                (nc.scalar if odma else nc.sync).dma_start(
                    out=o_d[:, goff:goff + gn], in_=ot[:])
    if not nc.is_finalized():
        nc.finalize()
    return nc


_CACHE = {}


def _get_nc(**kw):
    key = tuple(sorted(kw.items()))
    if key not in _CACHE:
        _CACHE[key] = build_bass(**kw)
    return _CACHE[key]


def kernel(verified_id, accept_lens, num_draft_tokens, **run_kw):
    from concourse.bass_utils import run_bass_kernel_spmd

    assert int(num_draft_tokens) == ND
    v = np.ascontiguousarray(np.asarray(verified_id, dtype=np.float32))
    lens = np.asarray(accept_lens)
    assert v.shape == (BS * ND,) and lens.shape == (BS,)

    build_kw = dict(BUILD_KW)
    lmode = build_kw.get("lens_i8", 1)
    if lmode in (4, 5, 6):
        import ml_dtypes

        npdt = {4: ml_dtypes.bfloat16, 5: np.float16, 6: ml_dtypes.float8_e4m3fn}[
            lmode
        ]
        l_np = np.ascontiguousarray(lens.astype(np.float32).astype(npdt))
    else:
        npdt = {0: np.int32, 1: np.int8, 2: np.int16, 3: np.uint8}[lmode]
        l_np = np.ascontiguousarray(lens.astype(npdt))

    v3 = v.reshape(N_CORES, P, FD_CORE)
    l3 = l_np.reshape(N_CORES, P, G_P)

    nc = _get_nc(**build_kw)
    in_maps = [{"v": v3[c], "lens": l3[c]} for c in range(N_CORES)]
    res = run_bass_kernel_spmd(nc, in_maps, core_ids=list(range(N_CORES)), **run_kw)
    out = np.stack([res.results[c]["o"] for c in range(N_CORES)])
    ret = out.reshape(-1)
    if run_kw:
        return ret, res
    return ret


BUILD_KW = {"lens_i8": 6}
